# revision 8
# baseline (speedup 1.0000x reference)
"""Trainium2 Bass kernel for ChemicalNet (per-species MLP / MoE routing).

Strategy
--------
Only atoms whose species is in {1, 6, 7, 8} produce output (others are 0),
and each such atom only needs ITS OWN species' 3-layer MLP.  The reference
runs all 4 expert networks on all atoms; we route on the host instead:

- host: map species -> expert index, collect per-expert atom index lists
- shard: 2 cores per expert, each core gets half of that expert's atoms
  (the per-core in_map carries that expert's weights, so the single SPMD
  program is expert-agnostic)
- host passes the gathered embedding columns TRANSPOSED ([128, n]) so the
  device needs no transposes: PE contracts over the partition axis directly
- device: L1 matmul+SiLU, L2 matmul (2-step K accum)+SiLU, L3 matmul -> [1,n]
- host scatters the compact per-core outputs back to the full [N, 1] output

Matmuls run in float16 (TensorEngine 1 cycle/col at any free size, vs
float32r's >=256-col requirement; ~5e-4 relative precision -- comfortably
inside the 2e-2 gate).  `dtype=` escape hatches build f32r / fp32 / bf16
programs.

DMA queue split (trace-driven): embeddings stream on the sync HWDGE queue
while w1/w2 ride the scalar HWDGE queue in parallel, so the first L1 matmul
isn't serialized behind the weight transfers.  Outputs also go out on the
sync HW queue -- the gpsimd SWDGE queue used previously adds ~1us of
software-descriptor latency right on the kernel's tail.  w3/biases (tiny,
needed late) stay on gpsimd.

The PE runs a DVFS ramp (0.65 -> 1.2 -> 2.4 GHz after ~3us of continuous
work), so a handful of throwaway matmuls on zeroed SBUF tiles are issued
while the input DMAs are still in flight: by the time real data lands the
array is already most of the way to full clock.

Per-chunk (512 atoms) the two 128-row halves of the hidden layer land in one
[128, 1024] PSUM tile so a single ACTIVATE applies SiLU to both (the scalar
engine does not pipeline ACTIVATEs; fewer/bigger is faster).  That merge
needs a bias constant along the free axis; biases in this problem are
identically zero, which the host verifies -- nonzero-bias inputs take a
(slower) per-half ACTIVATE path with per-partition bias.

The layer-3 [1, F] matmul accumulates into a corner of the layer-2 PSUM
tile after its ACTIVATE has read it (WAR handled by Tile), so all 8 PSUM
banks go to the 4-deep [128, 1024] pipeline pool.

All shapes are compile-time constants derived from the actual input
(the Bass program is built fresh per call).
"""

import numpy as np

import concourse.bass as bass
import concourse.tile as tile
from concourse import bacc, mybir
from concourse.bass_utils import run_bass_kernel_spmd

N_CORES = 8
NSPECIES = 4
SPECIES_Z = np.array([1, 6, 7, 8], dtype=np.int32)
MAXIDX = 118
D = 128          # embedding dim
H = 256          # hidden dim
F = 512          # atom-chunk size (one PSUM bank of fp32)
FP = mybir.dt.float32
SILU = mybir.ActivationFunctionType.Silu
N_WARM_MM = 0    # PE p-state warm-up matmuls issued while DMAs stream


def _chunk_sizes(npad: int) -> list[int]:
    """256-col head (f32r needs >=256 cols for full PE rate), 512s in the
    middle, and a small tail chunk so the last ACT->L3->copy->DMA chain is
    short."""
    sizes = []
    if npad > 256:
        sizes.append(256)
    while npad - sum(sizes) > F:
        sizes.append(F)
    rem = npad - sum(sizes)
    if rem > 384:
        h = (rem // 2 + 7) & ~7
        sizes += [h, rem - h]
    elif rem:
        sizes.append(rem)
    return sizes


def _build_program(npad: int, zero_bias: bool, mmdt):
    """One SPMD program: a 3-layer per-expert MLP over `npad` atom columns."""
    nc = bacc.Bacc("TRN2", target_bir_lowering=False, debug=False,
                   num_devices=N_CORES)

    embT_d = nc.dram_tensor("embT", [D, npad], mmdt, kind="ExternalInput")
    w1_d = nc.dram_tensor("w1", [D, H], mmdt, kind="ExternalInput")
    w2_d = nc.dram_tensor("w2", [2, 128, H], mmdt, kind="ExternalInput")
    w3_d = nc.dram_tensor("w3", [128, 2], mmdt, kind="ExternalInput")
    if not zero_bias:
        b1_d = nc.dram_tensor("b1", [128, 2], FP, kind="ExternalInput")
        b2_d = nc.dram_tensor("b2", [128, 2], FP, kind="ExternalInput")
        b3_d = nc.dram_tensor("b3", [1, 1], FP, kind="ExternalInput")
    out_d = nc.dram_tensor("out", [1, npad], FP, kind="ExternalOutput")

    sizes = _chunk_sizes(npad)
    chunks = []
    c0 = 0
    for s in sizes:
        chunks.append((c0, s))
        c0 += s
    nch = len(chunks)

    with tile.TileContext(nc) as tc:
        with (
            tc.tile_pool(name="singles", bufs=1) as singles,
            tc.tile_pool(name="emb", bufs=nch) as embp,
            tc.tile_pool(name="z1p", bufs=nch) as z1p,
            tc.tile_pool(name="z2p", bufs=nch) as z2p,
            tc.tile_pool(name="outp", bufs=3) as outp,
            tc.tile_pool(name="ps1", bufs=2, space="PSUM") as ps1p,
            tc.tile_pool(name="ps2", bufs=2, space="PSUM") as ps2p,
        ):
            # preload the SiLU table set while input DMAs run
            warm_act = singles.tile([128, 1], FP)
            nc.vector.memset(warm_act[:], 0.0)
            nc.scalar.activation(warm_act[:], warm_act[:], SILU)

            # --- input DMAs: emb chunks stream on the sync HW queue while
            # w1/w2 ride the scalar HW queue in parallel; w3/biases (tiny,
            # needed only at L3) go on the gpsimd SWDGE queue. ---
            emb_ts = []
            for ci, (c0, f) in enumerate(chunks):
                emb_c = embp.tile([D, F], mmdt, tag="emb", name=f"emb{ci}")
                emb_ts.append(emb_c)

            w1_t = singles.tile([D, H], mmdt)
            nc.scalar.dma_start(w1_t[:], w1_d[:])
            for ci, (c0, f) in enumerate(chunks):
                nc.sync.dma_start(emb_ts[ci][:, :f], embT_d[:, c0:c0 + f])
            w2_t = singles.tile([128, 2 * H], mmdt)
            for r in range(2):
                nc.scalar.dma_start(w2_t[:, r * H:(r + 1) * H], w2_d[r])
            w3_t = singles.tile([128, 2], mmdt)
            nc.gpsimd.dma_start(w3_t[:], w3_d[:])
            if not zero_bias:
                b1_t = singles.tile([128, 2], FP)
                nc.gpsimd.dma_start(b1_t[:], b1_d[:])
                b2_t = singles.tile([128, 2], FP)
                nc.gpsimd.dma_start(b2_t[:], b2_d[:])
                b3_t = singles.tile([1, 1], FP)
                nc.gpsimd.dma_start(b3_t[:], b3_d[:])

            # --- PE p-state warm-up: matmuls on zeroed tiles keep the array
            # busy from t~=0 so the DVFS ramp (full speed after ~3us of
            # continuous work) is mostly done when real data lands. ---
            if N_WARM_MM:
                # bf16 regardless of mmdt: memset of float32r is invalid ISA
                wdt = mybir.dt.bfloat16
                warm_w = singles.tile([128, 128], wdt)
                nc.vector.memset(warm_w[:], 0.0)
                warm_x = singles.tile([128, F], wdt)
                nc.vector.memset(warm_x[:], 0.0)
                warm_ps = ps1p.tile([128, 2 * F], FP, tag="ps1", name="warm_ps")
                for _ in range(N_WARM_MM):
                    nc.tensor.matmul(warm_ps[:, :F], warm_w[:], warm_x[:],
                                     start=True, stop=True)

            def m_off(f):
                # matmul output must stay inside one 512-col PSUM bank:
                # pack the m1 half right after m0 only when both fit bank 0
                return f if 2 * f <= F else F

            def act_pair(z_t, ps_t, f, b_t):
                """SiLU both m-halves of a psum tile -> z SBUF.

                Zero-bias path: ONE ACTIVATE over [0, m_off+f) -- for
                off==F > f this also covers the unused gap columns, which
                is harmless and cheaper than a second ACTIVATE."""
                off = m_off(f)
                if zero_bias:
                    nc.scalar.activation(z_t[:, :off + f], ps_t[:, :off + f],
                                         SILU)
                else:
                    for m in range(2):
                        nc.scalar.activation(
                            z_t[:, m * off:m * off + f],
                            ps_t[:, m * off:m * off + f], SILU,
                            bias=b_t[:, m:m + 1])

            # Software-pipelined emission over chunk PAIRS.  Within a pair,
            # the two chunks' matmuls that share a stationary weight tile are
            # emitted back-to-back, halving LDWEIGHTS traffic on the PE
            # queue (the weight reload between consecutive matmuls is the
            # single biggest non-column cost on the tensor engine).
            z1s, z2s, ps2s = {}, {}, {}

            def emit_l1(cis):
                for ci in cis:
                    c0, f = chunks[ci]
                    z1s[ci] = (
                        ps1p.tile([128, 2 * F], FP, tag="ps1", name=f"ps1_{ci}"),
                        z1p.tile([128, 2 * F], mmdt, tag="z1", name=f"z1_{ci}"),
                    )
                for m in range(2):
                    for ci in cis:
                        c0, f = chunks[ci]
                        off = m_off(f)
                        ps1 = z1s[ci][0]
                        nc.tensor.matmul(ps1[:, m * off:m * off + f],
                                         w1_t[:, m * 128:(m + 1) * 128],
                                         emb_ts[ci][:, :f],
                                         start=True, stop=True)
                for ci in cis:
                    c0, f = chunks[ci]
                    ps1, z1 = z1s[ci]
                    act_pair(z1, ps1, f, None if zero_bias else b1_t)

            def emit_l2(cis):
                for ci in cis:
                    ps2s[ci] = ps2p.tile([128, 2 * F], FP, tag="ps2",
                                        name=f"ps2_{ci}")
                for m in range(2):
                    for k in range(2):
                        for ci in cis:
                            c0, f = chunks[ci]
                            off = m_off(f)
                            z1 = z1s[ci][1]
                            nc.tensor.matmul(
                                ps2s[ci][:, m * off:m * off + f],
                                w2_t[:, k * H + m * 128:k * H + m * 128 + 128],
                                z1[:, k * off:k * off + f],
                                start=(k == 0), stop=(k == 1))
                for ci in cis:
                    c0, f = chunks[ci]
                    z2 = z2p.tile([128, 2 * F], mmdt, tag="z2", name=f"z2_{ci}")
                    act_pair(z2, ps2s[ci], f, None if zero_bias else b2_t)
                    z2s[ci] = z2

            def emit_l3(cis):
                # L3 accumulates into a corner of ps2 after its ACT read
                # (WAR handled by Tile) -- no extra PSUM bank needed.
                for k in range(2):
                    for ci in cis:
                        c0, f = chunks[ci]
                        off = m_off(f)
                        nc.tensor.matmul(ps2s[ci][0:1, 0:f],
                                         w3_t[:, k:k + 1],
                                         z2s[ci][:, k * off:k * off + f],
                                         start=(k == 0), stop=(k == 1))
                for ci in cis:
                    c0, f = chunks[ci]
                    out_t = outp.tile([1, F], FP, tag="out", name=f"out_{ci}")
                    if zero_bias:
                        nc.vector.tensor_copy(out_t[:, :f], ps2s[ci][0:1, 0:f])
                    else:
                        nc.vector.tensor_scalar_add(out_t[:, :f],
                                                    ps2s[ci][0:1, 0:f],
                                                    b3_t[0:1, 0:1])
                    nc.sync.dma_start(out_d[:, c0:c0 + f], out_t[:, :f])

            pairs = [tuple(range(i, min(i + 2, nch)))
                     for i in range(0, nch, 2)]
            npr = len(pairs)
            emit_l1(pairs[0])
            for p in range(npr):
                if p >= 1:
                    emit_l3(pairs[p - 1])
                emit_l2(pairs[p])
                if p + 1 < npr:
                    emit_l1(pairs[p + 1])
            emit_l3(pairs[npr - 1])

    nc.compile()
    return nc


def _route(species: np.ndarray):
    """species values -> expert idx (-1 unknown); per-core row assignments."""
    conv = np.full(MAXIDX + 2, -1, dtype=np.int32)
    conv[SPECIES_Z] = np.arange(NSPECIES, dtype=np.int32)
    idx = conv[species]
    core_rows = []
    for s in range(NSPECIES):
        rows = np.flatnonzero(idx == s)
        h = (len(rows) + 1) // 2
        core_rows.append(rows[:h])
        core_rows.append(rows[h:])
    return core_rows


_DTYPES = {
    "fp16": mybir.dt.float16,
    "bf16": mybir.dt.bfloat16,
    "f32r": mybir.dt.float32r,
    "fp32": mybir.dt.float32,
}


def _run(inputs: dict, trace: bool = False, dtype: str = "fp16"):
    species = inputs["species"]
    embedding = np.ascontiguousarray(inputs["embedding"], dtype=np.float32)
    n_atoms = species.shape[0]
    out_full = np.zeros((n_atoms, 1), dtype=np.float32)

    core_rows = _route(np.asarray(species))
    nmax = max(len(r) for r in core_rows)
    if nmax == 0:
        return out_full, None
    npad = -(-nmax // 8) * 8

    zero_bias = all(
        not np.any(np.asarray(inputs[k])) for k in ("b1", "b2", "b3"))
    mmdt = _DTYPES[dtype]
    np_mm = mybir.dt.np(mmdt)
    nc = _build_program(npad, zero_bias, mmdt)

    in_maps = []
    for c in range(N_CORES):
        s = c // 2
        rows = core_rows[c]
        embT = np.zeros((D, npad), dtype=np_mm)
        if len(rows):
            embT[:, :len(rows)] = embedding[rows].T.astype(np_mm)
        im = {
            "embT": embT,
            "w1": np.ascontiguousarray(
                np.asarray(inputs["W1"][s], dtype=np.float32).astype(np_mm)),
            "w2": np.ascontiguousarray(np.asarray(
                inputs["W2"][s], dtype=np.float32).reshape(2, 128, H).astype(np_mm)),
            "w3": np.ascontiguousarray(np.asarray(
                inputs["W3"][s], dtype=np.float32).reshape(2, 128).T.astype(np_mm)),
        }
        if not zero_bias:
            im["b1"] = np.ascontiguousarray(
                np.asarray(inputs["b1"][s], dtype=np.float32).reshape(2, 128).T)
            im["b2"] = np.ascontiguousarray(
                np.asarray(inputs["b2"][s], dtype=np.float32).reshape(2, 128).T)
            im["b3"] = np.asarray(inputs["b3"][s], dtype=np.float32).reshape(1, 1)
        in_maps.append(im)

    res = run_bass_kernel_spmd(nc, in_maps, core_ids=list(range(N_CORES)),
                               trace=trace)
    for c in range(N_CORES):
        rows = core_rows[c]
        if len(rows):
            out_full[rows, 0] = res.results[c]["out"][0, :len(rows)]
    return out_full, res


def kernel(**inputs) -> np.ndarray:
    out, _ = _run(inputs, trace=False)
    return out


# revision 10
# speedup vs baseline: 1.0211x; 1.0211x over previous
"""Trainium2 Bass kernel for ChemicalNet (per-species MLP / MoE routing).

Strategy
--------
Only atoms whose species is in {1, 6, 7, 8} produce output (others are 0),
and each such atom only needs ITS OWN species' 3-layer MLP.  The reference
runs all 4 expert networks on all atoms; we route on the host instead:

- host: map species -> expert index, collect per-expert atom index lists
- shard: 2 cores per expert, each core gets half of that expert's atoms
  (the per-core in_map carries that expert's weights, so the single SPMD
  program is expert-agnostic)
- host passes the gathered embedding columns TRANSPOSED ([128, n]) so the
  device needs no transposes: PE contracts over the partition axis directly
- device: L1 matmul+SiLU, L2 matmul (2-step K accum)+SiLU, L3 matmul -> [1,n]
- host scatters the compact per-core outputs back to the full [N, 1] output

Matmuls run in bfloat16 (measured fastest on hw: ~0.55 ns/col vs ~0.8 for
float32r/fp16 under the sustained-load clock throttle, and 2-byte LDWEIGHTS
halves the PE's weight-reload cost; ~4e-3 relative precision -- a 5x margin
inside the 2e-2 gate).  `dtype=` escape hatches build f32r / fp32 / fp16
programs.

DMA queue split (trace-driven): embeddings stream on the sync HWDGE queue
while w1/w2 ride the scalar HWDGE queue in parallel, so the first L1 matmul
isn't serialized behind the weight transfers.  Outputs also go out on the
sync HW queue -- the gpsimd SWDGE queue used previously adds ~1us of
software-descriptor latency right on the kernel's tail.  w3/biases (tiny,
needed late) stay on gpsimd.

The PE runs a DVFS ramp (0.65 -> 1.2 -> 2.4 GHz after ~3us of continuous
work), so a handful of throwaway matmuls on zeroed SBUF tiles are issued
while the input DMAs are still in flight: by the time real data lands the
array is already most of the way to full clock.

Per-chunk (512 atoms) the two 128-row halves of the hidden layer land in one
[128, 1024] PSUM tile so a single ACTIVATE applies SiLU to both (the scalar
engine does not pipeline ACTIVATEs; fewer/bigger is faster).  That merge
needs a bias constant along the free axis; biases in this problem are
identically zero, which the host verifies -- nonzero-bias inputs take a
(slower) per-half ACTIVATE path with per-partition bias.

The layer-3 [1, F] matmul accumulates into a corner of the layer-2 PSUM
tile after its ACTIVATE has read it (WAR handled by Tile), so all 8 PSUM
banks go to the 4-deep [128, 1024] pipeline pool.

All shapes are compile-time constants derived from the actual input
(the Bass program is built fresh per call).
"""

import numpy as np

import concourse.bass as bass
import concourse.tile as tile
from concourse import bacc, mybir
from concourse.bass_utils import run_bass_kernel_spmd

N_CORES = 8
NSPECIES = 4
SPECIES_Z = np.array([1, 6, 7, 8], dtype=np.int32)
MAXIDX = 118
D = 128          # embedding dim
H = 256          # hidden dim
F = 512          # atom-chunk size (one PSUM bank of fp32)
FP = mybir.dt.float32
SILU = mybir.ActivationFunctionType.Silu
N_WARM_MM = 0    # PE p-state warm-up matmuls issued while DMAs stream


def _chunk_sizes(npad: int) -> list[int]:
    """Ramped head chunks (small first chunk -> earliest possible first
    matmul while its DMA is small), 512s in the middle, and a small tail
    chunk so the last ACT->L3->copy->DMA chain is short."""
    sizes = []
    for s in (128, 256):
        if sum(sizes) + s <= npad:
            sizes.append(s)
    while npad - sum(sizes) > F:
        sizes.append(F)
    rem = npad - sum(sizes)
    if rem > 384:
        h = (rem // 2 + 7) & ~7
        sizes += [h, rem - h]
    elif rem:
        sizes.append(rem)
    return sizes


def _build_program(npad: int, zero_bias: bool, mmdt):
    """One SPMD program: a 3-layer per-expert MLP over `npad` atom columns."""
    nc = bacc.Bacc("TRN2", target_bir_lowering=False, debug=False,
                   num_devices=N_CORES)

    embT_d = nc.dram_tensor("embT", [D, npad], mmdt, kind="ExternalInput")
    w1_d = nc.dram_tensor("w1", [D, H], mmdt, kind="ExternalInput")
    w2_d = nc.dram_tensor("w2", [2, 128, H], mmdt, kind="ExternalInput")
    w3_d = nc.dram_tensor("w3", [128, 2], mmdt, kind="ExternalInput")
    if not zero_bias:
        b1_d = nc.dram_tensor("b1", [128, 2], FP, kind="ExternalInput")
        b2_d = nc.dram_tensor("b2", [128, 2], FP, kind="ExternalInput")
        b3_d = nc.dram_tensor("b3", [1, 1], FP, kind="ExternalInput")
    out_d = nc.dram_tensor("out", [1, npad], FP, kind="ExternalOutput")

    sizes = _chunk_sizes(npad)
    chunks = []
    c0 = 0
    for s in sizes:
        chunks.append((c0, s))
        c0 += s
    nch = len(chunks)

    with tile.TileContext(nc) as tc:
        with (
            tc.tile_pool(name="singles", bufs=1) as singles,
            tc.tile_pool(name="emb", bufs=nch) as embp,
            tc.tile_pool(name="z1p", bufs=nch) as z1p,
            tc.tile_pool(name="z2p", bufs=nch) as z2p,
            tc.tile_pool(name="outp", bufs=3) as outp,
            tc.tile_pool(name="ps1", bufs=2, space="PSUM") as ps1p,
            tc.tile_pool(name="ps2", bufs=2, space="PSUM") as ps2p,
        ):
            # preload the SiLU table set while input DMAs run
            warm_act = singles.tile([128, 1], FP)
            nc.vector.memset(warm_act[:], 0.0)
            nc.scalar.activation(warm_act[:], warm_act[:], SILU)

            # --- input DMAs: emb chunks stream on the sync HW queue while
            # w1/w2 ride the scalar HW queue in parallel; w3/biases (tiny,
            # needed only at L3) go on the gpsimd SWDGE queue. ---
            emb_ts = []
            for ci, (c0, f) in enumerate(chunks):
                emb_c = embp.tile([D, F], mmdt, tag="emb", name=f"emb{ci}")
                emb_ts.append(emb_c)

            w1_t = singles.tile([D, H], mmdt)
            nc.scalar.dma_start(w1_t[:], w1_d[:])
            for ci, (c0, f) in enumerate(chunks):
                nc.sync.dma_start(emb_ts[ci][:, :f], embT_d[:, c0:c0 + f])
            w2_t = singles.tile([128, 2 * H], mmdt)
            for r in range(2):
                nc.scalar.dma_start(w2_t[:, r * H:(r + 1) * H], w2_d[r])
            w3_t = singles.tile([128, 2], mmdt)
            nc.gpsimd.dma_start(w3_t[:], w3_d[:])
            if not zero_bias:
                b1_t = singles.tile([128, 2], FP)
                nc.gpsimd.dma_start(b1_t[:], b1_d[:])
                b2_t = singles.tile([128, 2], FP)
                nc.gpsimd.dma_start(b2_t[:], b2_d[:])
                b3_t = singles.tile([1, 1], FP)
                nc.gpsimd.dma_start(b3_t[:], b3_d[:])

            # --- PE p-state warm-up: matmuls on zeroed tiles keep the array
            # busy from t~=0 so the DVFS ramp (full speed after ~3us of
            # continuous work) is mostly done when real data lands. ---
            if N_WARM_MM:
                # bf16 regardless of mmdt: memset of float32r is invalid ISA
                wdt = mybir.dt.bfloat16
                warm_w = singles.tile([128, 128], wdt)
                nc.vector.memset(warm_w[:], 0.0)
                warm_x = singles.tile([128, F], wdt)
                nc.vector.memset(warm_x[:], 0.0)
                warm_ps = ps1p.tile([128, 2 * F], FP, tag="ps1", name="warm_ps")
                for _ in range(N_WARM_MM):
                    nc.tensor.matmul(warm_ps[:, :F], warm_w[:], warm_x[:],
                                     start=True, stop=True)

            def m_off(f):
                # matmul output must stay inside one 512-col PSUM bank:
                # pack the m1 half right after m0 only when both fit bank 0
                return f if 2 * f <= F else F

            def act_pair(z_t, ps_t, f, b_t):
                """SiLU both m-halves of a psum tile -> z SBUF.

                Zero-bias path: ONE ACTIVATE over [0, m_off+f) -- for
                off==F > f this also covers the unused gap columns, which
                is harmless and cheaper than a second ACTIVATE."""
                off = m_off(f)
                if zero_bias:
                    nc.scalar.activation(z_t[:, :off + f], ps_t[:, :off + f],
                                         SILU)
                else:
                    for m in range(2):
                        nc.scalar.activation(
                            z_t[:, m * off:m * off + f],
                            ps_t[:, m * off:m * off + f], SILU,
                            bias=b_t[:, m:m + 1])

            # Software-pipelined emission over chunk PAIRS.  Within a pair,
            # the two chunks' matmuls that share a stationary weight tile are
            # emitted back-to-back, halving LDWEIGHTS traffic on the PE
            # queue (the weight reload between consecutive matmuls is the
            # single biggest non-column cost on the tensor engine).
            z1s, z2s, ps2s = {}, {}, {}

            def emit_l1(cis):
                for ci in cis:
                    c0, f = chunks[ci]
                    z1s[ci] = (
                        ps1p.tile([128, 2 * F], FP, tag="ps1", name=f"ps1_{ci}"),
                        z1p.tile([128, 2 * F], mmdt, tag="z1", name=f"z1_{ci}"),
                    )
                for m in range(2):
                    for ci in cis:
                        c0, f = chunks[ci]
                        off = m_off(f)
                        ps1 = z1s[ci][0]
                        nc.tensor.matmul(ps1[:, m * off:m * off + f],
                                         w1_t[:, m * 128:(m + 1) * 128],
                                         emb_ts[ci][:, :f],
                                         start=True, stop=True)
                for ci in cis:
                    c0, f = chunks[ci]
                    ps1, z1 = z1s[ci]
                    act_pair(z1, ps1, f, None if zero_bias else b1_t)

            def emit_l2(cis):
                for ci in cis:
                    ps2s[ci] = ps2p.tile([128, 2 * F], FP, tag="ps2",
                                        name=f"ps2_{ci}")
                for m in range(2):
                    for k in range(2):
                        for ci in cis:
                            c0, f = chunks[ci]
                            off = m_off(f)
                            z1 = z1s[ci][1]
                            nc.tensor.matmul(
                                ps2s[ci][:, m * off:m * off + f],
                                w2_t[:, k * H + m * 128:k * H + m * 128 + 128],
                                z1[:, k * off:k * off + f],
                                start=(k == 0), stop=(k == 1))
                for ci in cis:
                    c0, f = chunks[ci]
                    z2 = z2p.tile([128, 2 * F], mmdt, tag="z2", name=f"z2_{ci}")
                    act_pair(z2, ps2s[ci], f, None if zero_bias else b2_t)
                    z2s[ci] = z2

            def emit_l3(cis):
                # L3 accumulates into a corner of ps2 after its ACT read
                # (WAR handled by Tile) -- no extra PSUM bank needed.
                for k in range(2):
                    for ci in cis:
                        c0, f = chunks[ci]
                        off = m_off(f)
                        nc.tensor.matmul(ps2s[ci][0:1, 0:f],
                                         w3_t[:, k:k + 1],
                                         z2s[ci][:, k * off:k * off + f],
                                         start=(k == 0), stop=(k == 1))
                for ci in cis:
                    c0, f = chunks[ci]
                    out_t = outp.tile([1, F], FP, tag="out", name=f"out_{ci}")
                    if zero_bias:
                        nc.vector.tensor_copy(out_t[:, :f], ps2s[ci][0:1, 0:f])
                    else:
                        nc.vector.tensor_scalar_add(out_t[:, :f],
                                                    ps2s[ci][0:1, 0:f],
                                                    b3_t[0:1, 0:1])
                    nc.sync.dma_start(out_d[:, c0:c0 + f], out_t[:, :f])

            pairs = [tuple(range(i, min(i + 2, nch)))
                     for i in range(0, nch, 2)]
            npr = len(pairs)
            emit_l1(pairs[0])
            for p in range(npr):
                if p >= 1:
                    emit_l3(pairs[p - 1])
                emit_l2(pairs[p])
                if p + 1 < npr:
                    emit_l1(pairs[p + 1])
            emit_l3(pairs[npr - 1])

    nc.compile()
    return nc


def _route(species: np.ndarray):
    """species values -> expert idx (-1 unknown); per-core row assignments."""
    conv = np.full(MAXIDX + 2, -1, dtype=np.int32)
    conv[SPECIES_Z] = np.arange(NSPECIES, dtype=np.int32)
    idx = conv[species]
    core_rows = []
    for s in range(NSPECIES):
        rows = np.flatnonzero(idx == s)
        h = (len(rows) + 1) // 2
        core_rows.append(rows[:h])
        core_rows.append(rows[h:])
    return core_rows


_DTYPES = {
    "fp16": mybir.dt.float16,
    "bf16": mybir.dt.bfloat16,
    "f32r": mybir.dt.float32r,
    "fp32": mybir.dt.float32,
}


def _run(inputs: dict, trace: bool = False, dtype: str = "bf16"):
    species = inputs["species"]
    embedding = np.ascontiguousarray(inputs["embedding"], dtype=np.float32)
    n_atoms = species.shape[0]
    out_full = np.zeros((n_atoms, 1), dtype=np.float32)

    core_rows = _route(np.asarray(species))
    nmax = max(len(r) for r in core_rows)
    if nmax == 0:
        return out_full, None
    npad = -(-nmax // 8) * 8

    zero_bias = all(
        not np.any(np.asarray(inputs[k])) for k in ("b1", "b2", "b3"))
    mmdt = _DTYPES[dtype]
    np_mm = mybir.dt.np(mmdt)
    nc = _build_program(npad, zero_bias, mmdt)

    in_maps = []
    for c in range(N_CORES):
        s = c // 2
        rows = core_rows[c]
        embT = np.zeros((D, npad), dtype=np_mm)
        if len(rows):
            embT[:, :len(rows)] = embedding[rows].T.astype(np_mm)
        im = {
            "embT": embT,
            "w1": np.ascontiguousarray(
                np.asarray(inputs["W1"][s], dtype=np.float32).astype(np_mm)),
            "w2": np.ascontiguousarray(np.asarray(
                inputs["W2"][s], dtype=np.float32).reshape(2, 128, H).astype(np_mm)),
            "w3": np.ascontiguousarray(np.asarray(
                inputs["W3"][s], dtype=np.float32).reshape(2, 128).T.astype(np_mm)),
        }
        if not zero_bias:
            im["b1"] = np.ascontiguousarray(
                np.asarray(inputs["b1"][s], dtype=np.float32).reshape(2, 128).T)
            im["b2"] = np.ascontiguousarray(
                np.asarray(inputs["b2"][s], dtype=np.float32).reshape(2, 128).T)
            im["b3"] = np.asarray(inputs["b3"][s], dtype=np.float32).reshape(1, 1)
        in_maps.append(im)

    res = run_bass_kernel_spmd(nc, in_maps, core_ids=list(range(N_CORES)),
                               trace=trace)
    for c in range(N_CORES):
        rows = core_rows[c]
        if len(rows):
            out_full[rows, 0] = res.results[c]["out"][0, :len(rows)]
    return out_full, res


def kernel(**inputs) -> np.ndarray:
    out, _ = _run(inputs, trace=False)
    return out
